# revision 10
# baseline (speedup 1.0000x reference)
"""Complex attention (split re/im softmax) on 8 trn2 NeuronCores.

Math per (b,h) pair (L=S=1024, E=D=64):
  scores_re[l,s] = sum_e qr[l,e]kr[s,e] + qi[l,e]ki[s,e]   (x 1/sqrt(E))
  scores_im[l,s] = sum_e qi[l,e]kr[s,e] - qr[l,e]ki[s,e]   (x 1/sqrt(E))
  Ar = softmax_s(scores_re); Ai = softmax_s(scores_im)
  Vre = Ar@vr - Ai@vi ; Vim = Ar@vi + Ai@vr

Kernel strategy (per core: 4 of the 32 (b,h) pairs):
  - Pack the re/im contraction into K=128 matmuls:
      qcat = [qr; qi]^T * scale   [128, L]
      kre  = [kr; ki]^T           [128, S]
      kim  = [-ki; kr]^T          [128, S]
    scoresT (s on partitions, l free) = kre_chunk.T @ qcat, kim_chunk.T @ qcat
    written as separate re/im PSUM slices [128, 1024] (2 banks each, 3 in
    rotation) so the tensor engine never waits long on exp draining PSUM.
  - exp: split across ScalarE (true Exp activation) and DVE (Schraudolph
    fast-exp: i16 = trunc(x*2^7/ln2 + magic) bit-cast as bf16; ~1.8% rms).
    Writing P^T as bf16. No max-subtraction (|scaled scores| < ~10).
  - AV: for each l-chunk of 128, accumulate over s-tiles into a single PSUM
    bank [128, 258]:
      av[:, 0:129]   += Pr^T_chunk.T @ [vr | vi | ones]
      av[:, 129:258] += Pi^T_chunk.T @ [vr | vi | ones]
    The ones column makes cols 128/257 the softmax denominators Zr/Zi.
  - No on-chip normalization: av goes PSUM -> SBUF bf16 (ScalarE/DVE copy,
    DMA can't read PSUM) -> DRAM; the host divides by Z and forms Vre/Vim
    during unshard (O(L*D) work, negligible).
  - Pipeline shaping against the TRN2 p-state + engine-balance model:
      * warmup/pacing dummy matmuls keep the PE continuously busy through
        the exp-drain-bound first pair so it ramps to the 2.4 GHz p-state;
      * pair 0's AV is split into two s-halves (partial sums merged on the
        host) so useful AV work exists before pair 0's exp fully drains;
      * exp slices and PSUM->SBUF copies are assigned per-pair to whichever
        of ScalarE/DVE has slack in that pipeline phase.
"""

import numpy as np
import ml_dtypes

import concourse.bass as bass
from concourse import mybir
from concourse.tile import TileContext
from concourse.bass_utils import run_bass_kernel_spmd

B, L, H, E = 4, 1024, 8, 64
S, D = 1024, 64
NCORES = 8
PAIRS = B * H              # 32 (b,h) pairs
PPC = PAIRS // NCORES      # 4 pairs per core
NT = S // 128              # 8 s-tiles
NL = L // 128              # 8 l-chunks
VW = 132                   # padded vaug width (vr 64 | vi 64 | ones 1 | pad 3)

BF16 = mybir.dt.bfloat16
F32 = mybir.dt.float32
I16 = mybir.dt.int16
AF = mybir.ActivationFunctionType
ALU = mybir.AluOpType

# Schraudolph fast-exp constants for bf16 output (see module docstring).
FEXP_A = 184.6649652          # 2^7 / ln 2
FEXP_B = 16256.0 - 7.5 + 0.5  # 127*2^7 - c, +0.5 compensates trunc-to-zero

# Which exp slices the DVE takes, per pair: set of (t, part); part 0=re 1=im.
# Pair 0 is drain-bound (no AV work yet) and pair 3's exp gates the final AV
# block, so both run 50/50; middle pairs lean on ScalarE (exact exp) to limit
# fast-exp error mass. Pair 1's phase is PE-light (only half of pair 0's AV
# runs there), so it gets 6 DVE slices to keep ScalarE under the phase time.
DVE_SLICES = [
    {(t, 1) for t in range(NT)},
    {(1, 1), (3, 1), (5, 1), (7, 1), (1, 0), (5, 0)},
    {(1, 1), (3, 1), (5, 1), (7, 1)},
    {(t, 1) for t in range(NT)},
]

W_START = 13   # warmup dummy matmuls before pair 0 (ramp the PE p-state)
W_PACE = 3     # pacing dummies per s-tile in pair 0's first half


def _split_excess_waits(nc, max_waits=1):
    """This toolchain's walrus accepts at most one sync wait per
    instruction; Tile's scheduler emits up to ~3. Move excess waits onto
    preceding same-engine nofuse NoOps (pure dispatch delay, semantics
    preserved)."""
    nsplit = 0
    for f in nc.m.functions:
        for blk in f.blocks:
            insts = list(blk.instructions)
            new = []
            changed = False
            for inst in insts:
                si = inst.sync_info
                if si is not None and si.on_wait and len(si.on_wait) > max_waits:
                    waits = list(si.on_wait)
                    excess = waits[:-max_waits]
                    for k in range(0, len(excess), max_waits):
                        nop = mybir.InstNoOp(
                            name=nc.get_next_instruction_name(), ins=[], outs=[]
                        )
                        nop.engine = inst.engine
                        nop.bass_nofuse = True
                        nop.sync_info = mybir.SyncInfo(
                            on_wait=excess[k : k + max_waits], on_update=[]
                        )
                        new.append(nop)
                        nsplit += 1
                    si.on_wait = waits[-max_waits:]
                    changed = True
                new.append(inst)
            if changed:
                blk.instructions = new
    return nsplit


def _build_program():
    nc = bass.Bass()
    qcat_d = nc.declare_dram_parameter("qcat", [PPC, 128, L], BF16, isOutput=False)
    kre_d = nc.declare_dram_parameter("kre", [PPC, 128, S], BF16, isOutput=False)
    kim_d = nc.declare_dram_parameter("kim", [PPC, 128, S], BF16, isOutput=False)
    vaug_d = nc.declare_dram_parameter("vaug", [PPC, 128, NT, VW], BF16, isOutput=False)
    # raw AV numerators + Z columns; host normalizes
    out_d = nc.declare_dram_parameter("out", [PPC, NL, 128, 258], BF16, isOutput=True)
    # pair 0's AV runs as two s-half partial sums; the first half lands here
    # and the host adds it into out[0] before normalizing
    outx_d = nc.declare_dram_parameter("outx", [NL, 128, 258], BF16, isOutput=True)

    with TileContext(nc) as tc:
        with (
            tc.tile_pool(name="io", bufs=3) as io,
            tc.tile_pool(name="pp", bufs=2 * NT) as pp,
            tc.tile_pool(name="ps", bufs=3, space="PSUM") as ps,
            tc.tile_pool(name="psa", bufs=2, space="PSUM") as psa,
            tc.tile_pool(name="ob", bufs=4) as ob,
            tc.tile_pool(name="wrm", bufs=1) as wrm,
        ):
            warm = wrm.tile([128, 258], BF16, tag="w")
            nc.gpsimd.memset(warm, 0)

            def emit_dummy():
                """A ~107ns matmul on zeros to keep the PE p-state ramped
                while real work is drain- or DMA-bound."""
                dps = psa.tile([128, 258], F32, tag="av")
                nc.tensor.matmul(
                    dps, lhsT=warm[:, 0:128], rhs=warm, start=True, stop=True
                )

            def emit_copy(av, o, engine):
                if engine == "sc":
                    nc.scalar.activation(out=o, in_=av, func=AF.Copy)
                else:
                    nc.vector.tensor_scalar(
                        out=o, in0=av, scalar1=0.0, scalar2=None, op0=ALU.add
                    )

            def emit_av(state, c, trange, dst, copy_engine, queue):
                """Partial AV over s-tiles `trange` + store for l-chunk c."""
                p_tiles, va_t = state
                av = psa.tile([128, 258], F32, tag="av")
                for i, t in enumerate(trange):
                    nc.tensor.matmul(
                        av[:, 0:129],
                        lhsT=p_tiles[t][:, c * 128 : (c + 1) * 128],
                        rhs=va_t[:, t, 0:129],
                        start=(i == 0),
                        stop=(i == len(trange) - 1),
                    )
                for i, t in enumerate(trange):
                    nc.tensor.matmul(
                        av[:, 129:258],
                        lhsT=p_tiles[t][:, L + c * 128 : L + (c + 1) * 128],
                        rhs=va_t[:, t, 0:129],
                        start=(i == 0),
                        stop=(i == len(trange) - 1),
                    )
                o = ob.tile([128, 258], BF16, tag="o")
                emit_copy(av, o, copy_engine)
                queue.dma_start(out=dst, in_=o)

            def emit_exp(ps_slice, p_slice, use_dve):
                if use_dve:
                    nc.vector.tensor_scalar(
                        out=p_slice.bitcast(I16),
                        in0=ps_slice,
                        scalar1=FEXP_A,
                        scalar2=FEXP_B,
                        op0=ALU.mult,
                        op1=ALU.add,
                    )
                else:
                    nc.scalar.activation(out=p_slice, in_=ps_slice, func=AF.Exp)

            def emit_scores(pair, t, q_t, kre_t, kim_t):
                ks = kre_t[:, t * 128 : (t + 1) * 128]
                ki = kim_t[:, t * 128 : (t + 1) * 128]
                p_t = pp.tile([128, 2 * L], BF16)  # re: 0:L, im: L:2L
                ps_re = ps.tile([128, L], F32, tag="s")
                for h in range(L // 512):
                    nc.tensor.matmul(
                        ps_re[:, h * 512 : (h + 1) * 512],
                        lhsT=ks, rhs=q_t[:, h * 512 : (h + 1) * 512],
                        start=True, stop=True,
                    )
                emit_exp(ps_re, p_t[:, 0:L], (t, 0) in DVE_SLICES[pair])
                ps_im = ps.tile([128, L], F32, tag="s")
                for h in range(L // 512):
                    nc.tensor.matmul(
                        ps_im[:, h * 512 : (h + 1) * 512],
                        lhsT=ki, rhs=q_t[:, h * 512 : (h + 1) * 512],
                        start=True, stop=True,
                    )
                emit_exp(ps_im, p_t[:, L : 2 * L], (t, 1) in DVE_SLICES[pair])
                return p_t

            def load_pair(pair):
                q_t = io.tile([128, L], BF16, tag="q")
                kre_t = io.tile([128, S], BF16, tag="kre")
                kim_t = io.tile([128, S], BF16, tag="kim")
                va_t = io.tile([128, NT, VW], BF16, tag="va")
                if pair == 0:
                    # Critical-path loads split in halves across both HWDGE
                    # queues so the first scores matmul (kre[:, :128] +
                    # q[:, :512]) starts ~2us earlier.
                    nc.sync.dma_start(out=kre_t[:, 0:512], in_=kre_d[pair][:, 0:512])
                    nc.scalar.dma_start(out=q_t[:, 0:512], in_=qcat_d[pair][:, 0:512])
                    nc.sync.dma_start(out=kre_t[:, 512:S], in_=kre_d[pair][:, 512:S])
                    nc.scalar.dma_start(out=q_t[:, 512:L], in_=qcat_d[pair][:, 512:L])
                    nc.sync.dma_start(out=kim_t, in_=kim_d[pair])
                    nc.scalar.dma_start(out=va_t, in_=vaug_d[pair])
                else:
                    nc.scalar.dma_start(out=q_t, in_=qcat_d[pair])
                    nc.scalar.dma_start(out=kre_t, in_=kre_d[pair])
                    nc.scalar.dma_start(out=kim_t, in_=kim_d[pair])
                    nc.scalar.dma_start(out=va_t, in_=vaug_d[pair])
                return q_t, kre_t, kim_t, va_t

            def out_queue(c):
                return nc.sync if c % 2 == 0 else nc.scalar

            # --- pair 0: special drain-bound phase -------------------------
            for _ in range(W_START):
                emit_dummy()
            q_t, kre_t, kim_t, va_t = load_pair(0)
            p_tiles0 = []
            for t in range(NT // 2):
                p_tiles0.append(emit_scores(0, t, q_t, kre_t, kim_t))
                for _ in range(W_PACE):
                    emit_dummy()
            state0 = (p_tiles0, va_t)
            for t in range(NT // 2, NT):
                p_tiles0.append(emit_scores(0, t, q_t, kre_t, kim_t))
                # two first-half partial-AV chunks per slot fill the PE while
                # exp of the second half drains
                for c in (2 * (t - NT // 2), 2 * (t - NT // 2) + 1):
                    emit_av(state0, c, range(NT // 2), outx_d[c],
                            "sc" if c % 2 == 0 else "dve", out_queue(c))
            prev = (p_tiles0, va_t, 0, range(NT // 2, NT))

            # --- steady pairs ---------------------------------------------
            COPY_ENG = {0: "dve", 1: "dve", 2: "sc", 3: "sc"}
            for pair in range(1, PPC):
                q_t, kre_t, kim_t, va_t = load_pair(pair)
                p_tiles = []
                for t in range(NT):
                    p_tiles.append(emit_scores(pair, t, q_t, kre_t, kim_t))
                    pp_tiles, pva_t, ppair, ptrange = prev
                    emit_av((pp_tiles, pva_t), t, ptrange, out_d[ppair, t],
                            COPY_ENG[ppair], out_queue(t))
                prev = (p_tiles, va_t, pair, range(NT))
            pp_tiles, pva_t, ppair, ptrange = prev
            for c in range(NL):
                emit_av((pp_tiles, pva_t), c, ptrange, out_d[ppair, c],
                        COPY_ENG[ppair], out_queue(c))

    _split_excess_waits(nc)
    return nc


_CACHED_NC = None


def _get_program():
    global _CACHED_NC
    if _CACHED_NC is None:
        _CACHED_NC = _build_program()
    return _CACHED_NC


def _prep_in_maps(inputs):
    return _prep(
        inputs["q_real"], inputs["q_imag"], inputs["k_real"], inputs["k_imag"],
        inputs["v_real"], inputs["v_imag"],
    )


def _prep(q_real, q_imag, k_real, k_imag, v_real, v_imag):
    bf16 = ml_dtypes.bfloat16
    scale = 1.0 / np.sqrt(E)

    # [B,L,H,E] -> [B,H,E,L]; pack re/im along E into 128 partitions
    qr_t = np.transpose(np.asarray(q_real, np.float32), (0, 2, 3, 1))
    qi_t = np.transpose(np.asarray(q_imag, np.float32), (0, 2, 3, 1))
    qcat = (np.concatenate([qr_t, qi_t], axis=2) * scale).astype(bf16)  # [B,H,128,L]

    kr_t = np.transpose(np.asarray(k_real, np.float32), (0, 2, 3, 1))
    ki_t = np.transpose(np.asarray(k_imag, np.float32), (0, 2, 3, 1))
    kre = np.concatenate([kr_t, ki_t], axis=2).astype(bf16)             # [B,H,128,S]
    kim = np.concatenate([-ki_t, kr_t], axis=2).astype(bf16)

    vr_t = np.transpose(np.asarray(v_real, np.float32), (0, 2, 1, 3))   # [B,H,S,D]
    vi_t = np.transpose(np.asarray(v_imag, np.float32), (0, 2, 1, 3))
    vaug = np.zeros((B, H, S, VW), np.float32)
    vaug[..., 0:D] = vr_t
    vaug[..., D : 2 * D] = vi_t
    vaug[..., 2 * D] = 1.0
    # [B,H,S,VW] -> [B,H,NT,128,VW] -> partition-major [B,H,128,NT,VW]
    vaug = np.transpose(vaug.reshape(B, H, NT, 128, VW), (0, 1, 3, 2, 4)).astype(bf16)

    qcat = qcat.reshape(PAIRS, 128, L)
    kre = kre.reshape(PAIRS, 128, S)
    kim = kim.reshape(PAIRS, 128, S)
    vaug = vaug.reshape(PAIRS, 128, NT, VW)

    in_maps = []
    for c in range(NCORES):
        sl = slice(c * PPC, (c + 1) * PPC)
        in_maps.append(
            {
                "qcat": np.ascontiguousarray(qcat[sl]),
                "kre": np.ascontiguousarray(kre[sl]),
                "kim": np.ascontiguousarray(kim[sl]),
                "vaug": np.ascontiguousarray(vaug[sl]),
            }
        )
    return in_maps


def kernel(q_real, q_imag, k_real, k_imag, v_real, v_imag, attn_mask=None):
    in_maps = _prep(q_real, q_imag, k_real, k_imag, v_real, v_imag)
    nc = _get_program()
    res = run_bass_kernel_spmd(nc, in_maps, list(range(NCORES)))
    outs = np.stack(
        [res.results[c]["out"].astype(np.float32) for c in range(NCORES)], axis=0
    )  # [8, PPC, NL, 128, 258]
    outx = np.stack(
        [res.results[c]["outx"].astype(np.float32) for c in range(NCORES)], axis=0
    )  # [8, NL, 128, 258]
    outs[:, 0] += outx  # pair 0 of each core was computed as two partial sums
    outs = outs.reshape(B, H, L, 258)
    pr = outs[..., 0:128]
    zr = outs[..., 128:129]
    pi = outs[..., 129:257]
    zi = outs[..., 257:258]
    v_re = pr[..., 0:D] / zr - pi[..., D : 2 * D] / zi     # [B,H,L,D]
    v_im = pr[..., D : 2 * D] / zr + pi[..., 0:D] / zi
    v_re = np.transpose(v_re, (0, 2, 1, 3))                # [B,L,H,D]
    v_im = np.transpose(v_im, (0, 2, 1, 3))
    return np.stack([v_re, v_im], axis=0).astype(np.float32)


# revision 36
# speedup vs baseline: 1.1095x; 1.1095x over previous
"""Complex attention (split re/im softmax) on 8 trn2 NeuronCores.

Math per (b,h) pair (L=S=1024, E=D=64):
  scores_re[l,s] = sum_e qr[l,e]kr[s,e] + qi[l,e]ki[s,e]   (x 1/sqrt(E))
  scores_im[l,s] = sum_e qi[l,e]kr[s,e] - qr[l,e]ki[s,e]   (x 1/sqrt(E))
  Ar = softmax_s(scores_re); Ai = softmax_s(scores_im)
  Vre = Ar@vr - Ai@vi ; Vim = Ar@vi + Ai@vr

Kernel strategy (per core: 4 of the 32 (b,h) pairs):
  - Pack the re/im contraction into K=128 matmuls:
      qcat = [qr; qi]^T * scale   [128, L]
      kre  = [kr; ki]^T           [128, S]
      kim  = [-ki; kr]^T          [128, S]
    scoresT (s on partitions, l free) = kre_chunk.T @ qcat, kim_chunk.T @ qcat
    written as separate re/im PSUM slices [128, 1024] (2 banks each, 3 in
    rotation) so the tensor engine never waits long on exp draining PSUM.
  - exp: split across ScalarE (true Exp activation) and DVE (Schraudolph
    fast-exp: i16 = trunc(x*2^7/ln2 + magic) bit-cast as bf16; ~1.8% rms).
    Writing P^T as bf16. No max-subtraction (|scaled scores| < ~10).
  - AV: for each l-chunk of 128, accumulate over s-tiles into a single PSUM
    bank [128, 258]:
      av[:, 0:129]   += Pr^T_chunk.T @ [vr | vi | ones]
      av[:, 129:258] += Pi^T_chunk.T @ [vr | vi | ones]
    The ones column makes cols 128/257 the softmax denominators Zr/Zi.
  - No on-chip normalization: av goes PSUM -> SBUF bf16 (ScalarE/DVE copy,
    DMA can't read PSUM) -> DRAM; the host divides by Z and forms Vre/Vim
    during unshard (O(L*D) work, negligible).
  - Pipeline shaping against the TRN2 p-state + engine-balance model:
      * warmup/pacing dummy matmuls keep the PE continuously busy through
        the exp-drain-bound first pair so it ramps to the 2.4 GHz p-state;
      * pair 0's AV is split into two s-halves (partial sums merged on the
        host) so useful AV work exists before pair 0's exp fully drains;
      * exp slices and PSUM->SBUF copies are assigned per-pair to whichever
        of ScalarE/DVE has slack in that pipeline phase.
"""

import numpy as np
import ml_dtypes

import concourse.bass as bass
from concourse import mybir
from concourse.tile import TileContext
from concourse.bass_utils import run_bass_kernel_spmd

B, L, H, E = 4, 1024, 8, 64
S, D = 1024, 64
NCORES = 8
PAIRS = B * H              # 32 (b,h) pairs
PPC = PAIRS // NCORES      # 4 pairs per core
NT = S // 128              # 8 s-tiles
NL = L // 128              # 8 l-chunks
VW = 132                   # padded vaug width (vr 64 | vi 64 | ones 1 | pad 3)

BF16 = mybir.dt.bfloat16
F32 = mybir.dt.float32
I16 = mybir.dt.int16
AF = mybir.ActivationFunctionType
ALU = mybir.AluOpType

# Schraudolph fast-exp constants for bf16 output (see module docstring).
FEXP_A = 184.6649652          # 2^7 / ln 2
FEXP_B = 16256.0 - 7.5 + 0.5  # 127*2^7 - c, +0.5 compensates trunc-to-zero

# Which exp slices the DVE takes, per pair: set of (t, part); part 0=re 1=im.
# Pair 0 is drain-bound (no AV work yet) and pair 3's exp gates the final AV
# block, so both run 50/50; middle pairs lean on ScalarE (exact exp) to limit
# fast-exp error mass. Pair 1's phase is PE-light (only half of pair 0's AV
# runs there), so it gets 6 DVE slices to keep ScalarE under the phase time.
import os

_DVE_N = [int(x) for x in os.environ.get("K_DVE", "8,5,5,8").split(",")]

def _dve_set(n):
    """First n of an im-first, odd-tile-first slice order."""
    order = [(1, 1), (3, 1), (5, 1), (7, 1), (0, 1), (2, 1), (4, 1), (6, 1),
             (1, 0), (3, 0), (5, 0), (7, 0), (0, 0), (2, 0), (4, 0), (6, 0)]
    return set(order[:n])

DVE_SLICES = [_dve_set(n) for n in _DVE_N]

W_START = int(os.environ.get("K_WSTART", "0"))
W_PACE = int(os.environ.get("K_WPACE", "2"))
W1_PACE = int(os.environ.get("K_W1PACE", "0"))
COPY_MODE = os.environ.get("K_COPY", "dd,ss")  # copies engine for pairs 0..3


def _split_excess_waits(nc, max_waits=1):
    """This toolchain's walrus accepts at most one sync wait per
    instruction; Tile's scheduler emits up to ~3. Move excess waits onto
    preceding same-engine nofuse NoOps (pure dispatch delay, semantics
    preserved)."""
    nsplit = 0
    for f in nc.m.functions:
        for blk in f.blocks:
            insts = list(blk.instructions)
            new = []
            changed = False
            for inst in insts:
                si = inst.sync_info
                if si is not None and si.on_wait and len(si.on_wait) > max_waits:
                    waits = list(si.on_wait)
                    excess = waits[:-max_waits]
                    for k in range(0, len(excess), max_waits):
                        nop = mybir.InstNoOp(
                            name=nc.get_next_instruction_name(), ins=[], outs=[]
                        )
                        nop.engine = inst.engine
                        nop.bass_nofuse = True
                        nop.sync_info = mybir.SyncInfo(
                            on_wait=excess[k : k + max_waits], on_update=[]
                        )
                        new.append(nop)
                        nsplit += 1
                    si.on_wait = waits[-max_waits:]
                    changed = True
                new.append(inst)
            if changed:
                blk.instructions = new
    return nsplit


def _build_program():
    nc = bass.Bass()
    qcat_d = nc.declare_dram_parameter("qcat", [PPC, 128, L], BF16, isOutput=False)
    kre_d = nc.declare_dram_parameter("kre", [PPC, 128, S], BF16, isOutput=False)
    kim_d = nc.declare_dram_parameter("kim", [PPC, 128, S], BF16, isOutput=False)
    vaug_d = nc.declare_dram_parameter("vaug", [PPC, 128, NT, VW], BF16, isOutput=False)
    # raw AV numerators + Z columns; host normalizes
    # each pair's AV is two s-half partial sums (A: s-tiles 0-3, B: 4-7),
    # summed on the host; this keeps every pipeline phase PE-bound.
    # Layout [pair, l-partition, chunk*258]: per-partition contiguous so one
    # 128-descriptor DMA ships a whole half-pair (the HWDGE queue charges
    # per descriptor, so 8 big stores beat 64 small ones).
    out_d = nc.declare_dram_parameter("out", [PPC, 128, NL * 258], BF16, isOutput=True)

    with TileContext(nc) as tc:
        with (
            tc.tile_pool(name="io", bufs=3) as io,
            tc.tile_pool(name="pp", bufs=2 * NT) as pp,
            tc.tile_pool(name="ps", bufs=int(os.environ.get("K_PS", "3")),
                         space="PSUM") as ps,
            tc.tile_pool(name="psa", bufs=int(os.environ.get("K_PSA", "2")),
                         space="PSUM") as psa,
            tc.tile_pool(name="ob", bufs=4) as ob,
            tc.tile_pool(name="wrm", bufs=1) as wrm,
        ):
            warm = wrm.tile([128, 258], BF16, tag="w")
            nc.gpsimd.memset(warm, 0)

            def emit_dummy():
                """A ~107ns matmul on zeros to keep the PE p-state ramped
                while real work is drain- or DMA-bound."""
                dps = psa.tile([128, 258], F32, tag="av")
                nc.tensor.matmul(
                    dps, lhsT=warm[:, 0:128], rhs=warm, start=True, stop=True
                )

            def emit_copy(av, o, engine):
                if engine == "both":
                    # halve the copy latency when both engines are idle
                    # (final AV block): ScalarE takes pr, DVE takes pi
                    nc.scalar.activation(
                        out=o[:, 0:129], in_=av[:, 0:129], func=AF.Copy
                    )
                    nc.vector.tensor_scalar(
                        out=o[:, 129:258], in0=av[:, 129:258],
                        scalar1=0.0, scalar2=None, op0=ALU.add,
                    )
                elif engine == "sc":
                    nc.scalar.activation(out=o, in_=av, func=AF.Copy)
                else:
                    nc.vector.tensor_scalar(
                        out=o, in0=av, scalar1=0.0, scalar2=None, op0=ALU.add
                    )

            def emit_av(state, c, trange, otile, copy_engine):
                """Partial AV over s-tiles `trange`, copied into `otile`'s
                chunk-c column range (the caller DMAs the whole half)."""
                p_tiles, va_t = state
                av = psa.tile([128, 258], F32, tag="av")
                for i, t in enumerate(trange):
                    nc.tensor.matmul(
                        av[:, 0:129],
                        lhsT=p_tiles[t][:, c * 128 : (c + 1) * 128],
                        rhs=va_t[:, t, 0:129],
                        start=(i == 0),
                        stop=(i == len(trange) - 1),
                    )
                for i, t in enumerate(trange):
                    nc.tensor.matmul(
                        av[:, 129:258],
                        lhsT=p_tiles[t][:, L + c * 128 : L + (c + 1) * 128],
                        rhs=va_t[:, t, 0:129],
                        start=(i == 0),
                        stop=(i == len(trange) - 1),
                    )
                emit_copy(av, otile[:, c * 258 : (c + 1) * 258], copy_engine)

            def emit_exp(ps_slice, p_slice, use_dve):
                if use_dve:
                    nc.vector.tensor_scalar(
                        out=p_slice.bitcast(I16),
                        in0=ps_slice,
                        scalar1=FEXP_A,
                        scalar2=FEXP_B,
                        op0=ALU.mult,
                        op1=ALU.add,
                    )
                else:
                    nc.scalar.activation(out=p_slice, in_=ps_slice, func=AF.Exp)

            def emit_scores(pair, t, q_t, kre_t, kim_t):
                ks = kre_t[:, t * 128 : (t + 1) * 128]
                ki = kim_t[:, t * 128 : (t + 1) * 128]
                p_t = pp.tile([128, 2 * L], BF16)  # re: 0:L, im: L:2L
                ps_re = ps.tile([128, L], F32, tag="s")
                for h in range(L // 512):
                    nc.tensor.matmul(
                        ps_re[:, h * 512 : (h + 1) * 512],
                        lhsT=ks, rhs=q_t[:, h * 512 : (h + 1) * 512],
                        start=True, stop=True,
                    )
                emit_exp(ps_re, p_t[:, 0:L], (t, 0) in DVE_SLICES[pair])
                ps_im = ps.tile([128, L], F32, tag="s")
                for h in range(L // 512):
                    nc.tensor.matmul(
                        ps_im[:, h * 512 : (h + 1) * 512],
                        lhsT=ki, rhs=q_t[:, h * 512 : (h + 1) * 512],
                        start=True, stop=True,
                    )
                emit_exp(ps_im, p_t[:, L : 2 * L], (t, 1) in DVE_SLICES[pair])
                return p_t

            def load_pair(pair):
                q_t = io.tile([128, L], BF16, tag="q")
                kre_t = io.tile([128, S], BF16, tag="kre")
                kim_t = io.tile([128, S], BF16, tag="kim")
                va_t = io.tile([128, NT, VW], BF16, tag="va")
                if pair == 0:
                    # Critical-path loads split across both HWDGE queues in
                    # first-use order: k-tensors on SP, q on Act, va last
                    # (first needed ~14us in, at pair 0's first AV chunk).
                    nc.sync.dma_start(out=kre_t[:, 0:512], in_=kre_d[pair][:, 0:512])
                    nc.scalar.dma_start(out=q_t[:, 0:512], in_=qcat_d[pair][:, 0:512])
                    nc.sync.dma_start(out=kim_t[:, 0:512], in_=kim_d[pair][:, 0:512])
                    nc.scalar.dma_start(out=q_t[:, 512:L], in_=qcat_d[pair][:, 512:L])
                    nc.sync.dma_start(out=kre_t[:, 512:S], in_=kre_d[pair][:, 512:S])
                    nc.sync.dma_start(out=kim_t[:, 512:S], in_=kim_d[pair][:, 512:S])
                    nc.sync.dma_start(out=va_t, in_=vaug_d[pair])
                else:
                    # inputs stay on the SP queue: the Act queue carries
                    # output stores, whose not-yet-satisfied waits would
                    # head-of-line-block later input loads behind them
                    nc.sync.dma_start(out=q_t, in_=qcat_d[pair])
                    nc.sync.dma_start(out=kre_t, in_=kre_d[pair])
                    nc.sync.dma_start(out=kim_t, in_=kim_d[pair])
                    nc.sync.dma_start(out=va_t, in_=vaug_d[pair])
                return q_t, kre_t, kim_t, va_t

            def out_queue(c):
                return nc.scalar

            # Schedule: pair p's slots t=0..3 run scores(p,t) + B-half AV
            # chunks of pair p-1 (s-tiles 4-7); slots t=4..7 run scores(p,t)
            # + A-half AV chunks of pair p (s-tiles 0-3, whose exp has just
            # drained). Every slot is ~1.7us of PE work vs ~1.3us of exp
            # drain, so the tensor engine stays the bottleneck throughout.
            def copy_eng(parity_split, c):
                # steady-state copies ride the (exp-light) DVE; where both
                # engines have slack, alternate to halve rotation latency
                if parity_split:
                    return "dve" if c % 2 == 0 else "sc"
                return "dve"

            for _ in range(W_START):
                emit_dummy()
            prev = None
            for pair in range(PPC):
                q_t, kre_t, kim_t, va_t = load_pair(pair)
                p_tiles = []
                ob_t = None
                if prev is not None:
                    ob_t = ob.tile([128, NL * 258], BF16, tag="ob", name="ob_t")
                for t in range(NT):
                    p_tiles.append(emit_scores(pair, t, q_t, kre_t, kim_t))
                    if prev is not None:
                        pp_tiles, pva_t, ppair = prev
                        emit_av((pp_tiles, pva_t), t, range(NT), ob_t,
                                copy_eng(ppair == PPC - 1, t))
                    elif W_PACE:
                        for _ in range(W_PACE):
                            emit_dummy()
                if prev is not None:
                    nc.scalar.dma_start(out=out_d[prev[2]], in_=ob_t)
                prev = (p_tiles, va_t, pair)
            # final pair: quarter the output DMA so the store pipeline
            # drains concurrently with the last AV chunks (shorter tail)
            ob_f = ob.tile([128, NL * 258], BF16, tag="ob", name="ob_f")
            for c in range(NL):
                emit_av((prev[0], prev[1]), c, range(NT), ob_f,
                        copy_eng(True, c))
                if c % 2 == 1:
                    # SP queue: empty by now, so the tail never waits behind
                    # pair 2's big store on the Act queue
                    nc.sync.dma_start(
                        out=out_d[prev[2]][:, (c - 1) * 258 : (c + 1) * 258],
                        in_=ob_f[:, (c - 1) * 258 : (c + 1) * 258],
                    )

    _split_excess_waits(nc)
    return nc


_CACHED_NC = None


def _get_program():
    global _CACHED_NC
    if _CACHED_NC is None:
        _CACHED_NC = _build_program()
    return _CACHED_NC


def _prep_in_maps(inputs):
    return _prep(
        inputs["q_real"], inputs["q_imag"], inputs["k_real"], inputs["k_imag"],
        inputs["v_real"], inputs["v_imag"],
    )


def _prep(q_real, q_imag, k_real, k_imag, v_real, v_imag):
    bf16 = ml_dtypes.bfloat16
    scale = 1.0 / np.sqrt(E)

    # [B,L,H,E] -> [B,H,E,L]; pack re/im along E into 128 partitions
    qr_t = np.transpose(np.asarray(q_real, np.float32), (0, 2, 3, 1))
    qi_t = np.transpose(np.asarray(q_imag, np.float32), (0, 2, 3, 1))
    qcat = (np.concatenate([qr_t, qi_t], axis=2) * scale).astype(bf16)  # [B,H,128,L]

    kr_t = np.transpose(np.asarray(k_real, np.float32), (0, 2, 3, 1))
    ki_t = np.transpose(np.asarray(k_imag, np.float32), (0, 2, 3, 1))
    kre = np.concatenate([kr_t, ki_t], axis=2).astype(bf16)             # [B,H,128,S]
    kim = np.concatenate([-ki_t, kr_t], axis=2).astype(bf16)

    vr_t = np.transpose(np.asarray(v_real, np.float32), (0, 2, 1, 3))   # [B,H,S,D]
    vi_t = np.transpose(np.asarray(v_imag, np.float32), (0, 2, 1, 3))
    vaug = np.zeros((B, H, S, VW), np.float32)
    vaug[..., 0:D] = vr_t
    vaug[..., D : 2 * D] = vi_t
    vaug[..., 2 * D] = 1.0
    # [B,H,S,VW] -> [B,H,NT,128,VW] -> partition-major [B,H,128,NT,VW]
    vaug = np.transpose(vaug.reshape(B, H, NT, 128, VW), (0, 1, 3, 2, 4)).astype(bf16)

    qcat = qcat.reshape(PAIRS, 128, L)
    kre = kre.reshape(PAIRS, 128, S)
    kim = kim.reshape(PAIRS, 128, S)
    vaug = vaug.reshape(PAIRS, 128, NT, VW)

    in_maps = []
    for c in range(NCORES):
        sl = slice(c * PPC, (c + 1) * PPC)
        in_maps.append(
            {
                "qcat": np.ascontiguousarray(qcat[sl]),
                "kre": np.ascontiguousarray(kre[sl]),
                "kim": np.ascontiguousarray(kim[sl]),
                "vaug": np.ascontiguousarray(vaug[sl]),
            }
        )
    return in_maps


def kernel(q_real, q_imag, k_real, k_imag, v_real, v_imag, attn_mask=None):
    in_maps = _prep(q_real, q_imag, k_real, k_imag, v_real, v_imag)
    nc = _get_program()
    res = run_bass_kernel_spmd(nc, in_maps, list(range(NCORES)))
    outs = np.stack(
        [res.results[c]["out"].astype(np.float32) for c in range(NCORES)],
        axis=0,
    )  # [8, PPC, 128, NL*258]
    # [core*pair, l_in_chunk(partition), chunk, 258] -> [.., chunk, l, 258]
    outs = outs.reshape(NCORES * PPC, 128, NL, 258).transpose(0, 2, 1, 3)
    outs = np.ascontiguousarray(outs).reshape(B, H, L, 258)
    pr = outs[..., 0:128]
    zr = outs[..., 128:129]
    pi = outs[..., 129:257]
    zi = outs[..., 257:258]
    v_re = pr[..., 0:D] / zr - pi[..., D : 2 * D] / zi     # [B,H,L,D]
    v_im = pr[..., D : 2 * D] / zr + pi[..., 0:D] / zi
    v_re = np.transpose(v_re, (0, 2, 1, 3))                # [B,L,H,D]
    v_im = np.transpose(v_im, (0, 2, 1, 3))
    return np.stack([v_re, v_im], axis=0).astype(np.float32)
